# revision 11
# baseline (speedup 1.0000x reference)
"""Trainium2 Bass kernel for nn_Biholomorphic_k8 — v5.

zzbar(i,j) = zz_i * conj(zz_j), zz = the 495 degree-8 monomials of z in C^5.
Device computes the pair products via 4-way row-tiled K=2 matmuls
(32x128 PE tiling: batches 4q+m run concurrently on tiles T0/T4/T8/T12),
evacuates PSUM->SBUF alternating ACT/DVE (greedy-balanced), and streams the
bf16 blob to HBM in staged DMAs on alternating HWDGE queues.

The 495-entry monomial table (0.01% of the FLOPs) is computed host-side and
shipped pre-arranged in the matmul operand layouts:
  rhs  ZT : partition 32m+r holds (re|im)[r] of zz[b] for b=4q+m at col 495q+j
  lhsT ZTc: partition 32m+r holds merged [re|im]/[im|-re] halves per i-block h
Per (seg h, quad q, tile m): out[Mh, Nh] = lhsT[2, Mh].T @ rhs[2, Nh] in PSUM;
rows 0..half-1 = re part of zzbar(i,:), rows half.. = im part.
Host unpacks the blob (drops the lower-triangle junk) exactly as v4.
"""

import itertools
import math
import os
import sys

import numpy as np

if "/opt/trn_rl_repo" not in sys.path:
    sys.path.insert(0, "/opt/trn_rl_repo")

N_COORD = 5
DEGREE = 8
N_MONO = 495
N_PAIRS = 122760
OUT_W = 245025
B = 256
B_CORE = 32
N_CORES = 8

MONOMIAL_IDX = np.array(
    list(itertools.combinations_with_replacement(range(N_COORD), DEGREE)),
    dtype=np.int32)                      # [495, 8]

OFF_RE = np.concatenate([[0], np.cumsum(495 - np.arange(495))]).astype(np.int64)
OFF_IM = np.concatenate([[0], np.cumsum(494 - np.arange(494))]).astype(np.int64)

H_HALF = [64] * 7 + [47]              # i-half height per h
H_M = [2 * x for x in H_HALF]         # out partitions
H_N = [495 - 64 * h for h in range(8)]
H_OFF = [128 * h for h in range(7)] + [896]   # ZTc col offset of block h

# seg = (h, batch-pair p): 2 matmuls on PE tiles {2(p%2), 2(p%2)+1}, 2 PSUM
# banks, pool bufs=4 -> 4 segs in flight so ACT/DVE evacs fully overlap.
# narrow h first so the first stages fill fast
SEGS = [(h, p) for h in range(7, -1, -1) for p in range(16)]
SEG_W = [2 * H_N[h] for h, _ in SEGS]
BTOT = sum(SEG_W)                     # 69376

# stage schedule: ramp up so the blob DMA stream starts early (the DMA queue
# is the end-to-end pacer: it runs saturated from first issue to the end)
STAGE_CAPS = [512, 1024, 2048, 4096, 8192, 16384, 16384, 16384, 4352]
assert sum(STAGE_CAPS) == BTOT
STAGE_MAX = max(STAGE_CAPS)

# greedy ACT/DVE balance for the evacuation copies (HW-measured coefficients:
# ACT (w+352)/1.2 ns, DVE 1.132*w+70 ns)
EVAC_ENG = []
_t_act = _t_dve = 0.0
for _w in SEG_W:
    _ca = (_w + 352) / 1.2
    _cd = 1.132 * _w + 70.0
    if _t_act + _ca <= _t_dve + _cd:
        EVAC_ENG.append("act"); _t_act += _ca
    else:
        EVAC_ENG.append("dve"); _t_dve += _cd


def _build_unpack():
    rowm = np.full((128, BTOT), -1, dtype=np.int32)
    colm = np.zeros((128, BTOT), dtype=np.int64)
    x0 = 0
    for (h, p), w in zip(SEGS, SEG_W):
        half, Nh = H_HALF[h], H_N[h]
        j = 64 * h + np.arange(Nh)
        rowv = np.full((128, Nh), -1, dtype=np.int32)
        colv = np.zeros((128, Nh), dtype=np.int64)
        for pr in range(2 * half):
            if pr < half:
                i = 64 * h + pr
                v = j >= i
                c = OFF_RE[i] + (j - i)
            else:
                i = 64 * h + (pr - half)
                v = j > i
                c = N_PAIRS + OFF_IM[min(i, 493)] + (j - i - 1)
            rowv[pr, v] = 0
            colv[pr, v] = c[v]
        for mp in range(2):
            b_ = 2 * p + mp
            sl = slice(x0 + mp * Nh, x0 + (mp + 1) * Nh)
            rowm[:, sl] = np.where(rowv >= 0, b_, -1)
            colm[:, sl] = colv
        x0 += w
    assert x0 == BTOT
    return rowm, colm


_ROWM, _COLM = _build_unpack()

# host-side ZTc column maps: col cc in [0, 990) of lhsT row r ->
# (index into sep[b] = [re(495) | im(495)], sign)
_M2 = np.zeros((2, 990), dtype=np.int64)
_S2 = np.ones((2, 990), dtype=np.float32)
for _h in range(8):
    _half = H_HALF[_h]
    _jj = np.arange(_half)
    _i = 64 * _h + _jj
    for _T in range(2):
        _cc = H_OFF[_h] + _half * _T + _jj
        # r=0: T=0 -> re(i), T=1 -> im(i);  r=1: T=0 -> im(i), T=1 -> -re(i)
        _M2[0, _cc] = _i if _T == 0 else 495 + _i
        _M2[1, _cc] = (495 + _i) if _T == 0 else _i
        if _T == 1:
            _S2[1, _cc] = -1.0

_PROGRAM = None


def _build_program():
    import concourse.bacc as bacc
    import concourse.mybir as mybir
    from concourse.tile import TileContext
    from concourse.ap import AP

    f32 = mybir.dt.float32
    bf16 = mybir.dt.bfloat16

    nc = bacc.Bacc(None)
    ztin = nc.dram_tensor("ztin", [8, 3960], bf16, kind="ExternalInput")
    ztcin = nc.dram_tensor("ztcin", [8, 7920], bf16, kind="ExternalInput")
    blob = nc.dram_tensor("blob", [128, BTOT], bf16, kind="ExternalOutput")

    with TileContext(nc) as tc:
        with (
            tc.tile_pool(name="const", bufs=1) as cpool,
            tc.tile_pool(name="stage", bufs=3) as opool,
            tc.tile_pool(name="bp", bufs=4, space="PSUM") as bpool,
        ):
            ZT = cpool.tile([128, 3960], bf16)
            ZTc = cpool.tile([128, 7920], bf16)

            # gather loads: DRAM rows {2m, 2m+1} -> SBUF partitions {32m, 32m+1}
            # (sync + gpsimd queues only -- keep ACT free for early evacs)
            engs = (nc.sync, nc.gpsimd)
            qi = 0
            for m in range(4):
                for sbt, dram, W in ((ZT, ztin, 3960), (ZTc, ztcin, 7920)):
                    engs[qi % 2].dma_start(sbt[32 * m:32 * m + 2, 0:W],
                                           dram[2 * m:2 * m + 2, :])
                    qi += 1

            stage = {"t": None, "o": 0, "off": 0, "i": 0}

            def stage_flush():
                eng = nc.sync if stage["i"] % 2 == 0 else nc.gpsimd
                eng.dma_start(
                    blob[:, stage["off"]:stage["off"] + stage["o"]],
                    stage["t"][:, 0:stage["o"]])
                stage["off"] += stage["o"]
                stage["i"] += 1
                stage["t"] = None

            def stage_alloc(w):
                cap = STAGE_CAPS[min(stage["i"], len(STAGE_CAPS) - 1)]
                if stage["t"] is not None and stage["o"] + w > cap:
                    stage_flush()
                if stage["t"] is None:
                    stage["t"] = opool.tile([128, STAGE_MAX], bf16,
                                            name="S", tag="S")
                    stage["o"] = 0
                t, o = stage["t"], stage["o"]
                stage["o"] += w
                return t, o

            for si, (h, p) in enumerate(SEGS):
                Mh, Nh, off = H_M[h], H_N[h], H_OFF[h]
                q = p // 2
                pt = bpool.tile([128, 1024], f32, tag="bp")
                for mp in range(2):
                    m = 2 * (p % 2) + mp
                    nc.tensor.matmul(
                        pt[0:Mh, 512 * mp:512 * mp + Nh],
                        ZTc[32 * m:32 * m + 2,
                            990 * q + off:990 * q + off + Mh],
                        ZT[32 * m:32 * m + 2,
                           495 * q + 64 * h:495 * q + 495],
                        start=True, stop=True, tile_position=(32 * m, 0))
                sl = pt[:, 0:1024]
                sap = AP(sl.tensor, sl.offset,
                         [list(sl.ap[0]), [512, 2], [1, Nh]])
                w = 2 * Nh
                t, o = stage_alloc(w)
                dsl = t[:, o:o + w]
                dap = AP(dsl.tensor, dsl.offset,
                         [list(dsl.ap[0]), [Nh, 2], [1, Nh]])
                if EVAC_ENG[si] == "dve":
                    nc.vector.tensor_copy(dap, sap)
                else:
                    nc.scalar.copy(dap, sap)

            if stage["t"] is not None:
                stage_flush()
            assert stage["off"] == BTOT, (stage["off"], BTOT)

    nc.compile()
    return nc


def _get_program():
    global _PROGRAM
    if _PROGRAM is None:
        _PROGRAM = _build_program()
    return _PROGRAM


LAST_EXEC_NS = None


def kernel(z_re: np.ndarray, z_im: np.ndarray) -> np.ndarray:
    global LAST_EXEC_NS
    import ml_dtypes
    from concourse.bass_utils import run_bass_kernel_spmd

    z_re = np.asarray(z_re, dtype=np.float32)
    z_im = np.asarray(z_im, dtype=np.float32)
    assert z_re.shape == (B, N_COORD) and z_im.shape == (B, N_COORD)

    nc = _get_program()

    # host: degree-8 monomials (tiny), pre-arranged operand tables
    z = z_re.astype(np.complex64) + 1j * z_im.astype(np.complex64)
    zz = np.prod(z[:, MONOMIAL_IDX], axis=-1)          # [256, 495] c64
    sep = np.concatenate([zz.real, zz.imag], axis=1)   # [256, 990] f32
    bf = ml_dtypes.bfloat16

    in_maps = []
    for c in range(N_CORES):
        s = sep[c * B_CORE:(c + 1) * B_CORE]           # [32, 990]
        # zt: [q, m, r, j] -> [m, r, q, j] -> [8, 3960]
        zt = (s.reshape(8, 4, 2, 495)
              .transpose(1, 2, 0, 3).reshape(8, 3960).astype(bf))
        # ztc: gather per lhsT row r then arrange [m, r, q, cc]
        g = np.stack([s[:, _M2[0]] * _S2[0],
                      s[:, _M2[1]] * _S2[1]], axis=1)  # [32, 2, 990]
        ztc = (g.reshape(8, 4, 2, 990)
               .transpose(1, 2, 0, 3).reshape(8, 7920).astype(bf))
        in_maps.append({"ztin": zt, "ztcin": ztc})

    trace = bool(os.environ.get("BIHOLO_TRACE"))
    res = run_bass_kernel_spmd(
        nc, in_maps, core_ids=list(range(N_CORES)), trace=trace)
    if trace:
        LAST_EXEC_NS = res.exec_time_ns

    valid = _ROWM >= 0
    rows_v = _ROWM[valid]
    cols_v = _COLM[valid]
    out = np.empty((B, OUT_W), np.float32)
    for c in range(N_CORES):
        bl = np.asarray(res.results[c]["blob"]).astype(np.float32)
        out[B_CORE * c + rows_v, cols_v] = bl[valid]
    return out
